# revision 21
# baseline (speedup 1.0000x reference)
"""Trainium2 Bass kernel for the CCQC quantum-circuit classifier.

The whole circuit (one layer: RX/RZ/RX per qubit, then CPhase+RX ring) is a
fixed linear operator on the 1024-dim state vector.  On the host we fold all
40 gates into a single 1024x1024 complex matrix M (cheap: ~1s of numpy on
2x1024x1024 floats), so that for a batch row xf:

    state_final = xf @ M            (xf real, M complex)
    probs       = |state_final|^2
    out         = (probs @ signsT) / ||xf||^2

The division uses unitarity: sum_j probs[j] = ||xf||^2, so an extra ones
column appended to signsT yields the normalizer for free.

Device work per core (batch 512 of 4096):
    RE^T = M_re^T x^T, IM^T = M_im^T x^T   (TensorE, K=1024 contraction)
    probsT = RE^2 + IM^2                    (ScalarE square + VectorE add)
    outT   = [signs|1]^T probsT             (TensorE, contraction over 1024)
    out    = outT[:,0:10] * recip(outT[:,10])

The walrus build in this container allows AT MOST ONE sync-wait per
instruction.  All cross-engine dependencies are therefore funneled through
explicit single-wait NOP "gates" (nofuse=True) wired with add_dep_helper;
each real instruction is left with at most one unobserved semaphore.
"""

import numpy as np

import concourse.bass as bass
import concourse.tile as tile
from concourse import mybir
from concourse.bass_utils import run_bass_kernel_spmd
from concourse.tile_rust import add_dep_helper
from concourse.vector_clock import ScopedClock, VectorClock

# The walrus build here accepts at most ONE sync wait per instruction, but
# Tile's kernel-tail emits a single Drain waiting on every proc's semaphore.
# Split that into a chain of single-wait pre-drains (one proc each); the
# final stock drain then finds everything already observed and gets no waits.
from concourse.tile_sem_assignment import tick_to_sem


def _split_drain_and_barrier(self, tick_clock, wait_clock):
    ticks = eval(repr(tick_clock.global_clock)
                 .replace("VectorClock(", "").rstrip(")"))
    allocated = dict(wait_clock.sems.allocated())
    for p, t in enumerate(ticks):
        if t > 0 and p in allocated:
            self.nc.sync.wait_ge(allocated[p], tick_to_sem(t, p))
    self.nc.sync.drain()
    self.nc.all_engine_barrier()
    popped = self.nc._tile_sem_poison_stack.pop()
    assert popped is self._sem_poison
    self.nc.clear_and_free_semaphores(list(self.sems.allocated().values()))
    self.nc.all_engine_barrier()


tile.TileContext._drain_and_barrier = _split_drain_and_barrier

N_CORES = 8
N_QUBITS = 10
DIM = 1 << N_QUBITS          # 1024
B = 4096
BS = B // N_CORES            # 512 rows per core
KT = DIM // 128              # 8 contraction tiles
JT = DIM // 128              # 8 output-column tiles
NAUG = N_QUBITS + 1          # signs columns + ones column

# matmul dtype for the two big (512x1024x1024) products:
#   float32  = exact, 4 cycles/row on the PE
#   float32r = TF32-like reduced precision, 1 cycle/row (4x faster)
MM_DTYPE = mybir.dt.float32r


# ----------------------------------------------------------------- host math

def _build_circuit_matrix(weights):
    """M (DIM, DIM) complex128 with final_state_row = xf_row @ M."""
    w = np.asarray(weights, dtype=np.float64)
    M = np.eye(DIM, dtype=np.complex128)

    def apply_1q(state, U, wire):
        left = 1 << wire
        right = 1 << (N_QUBITS - 1 - wire)
        s = state.reshape(-1, left, 2, right)
        s0 = s[:, :, 0, :]
        s1 = s[:, :, 1, :]
        out = np.empty_like(s)
        out[:, :, 0, :] = U[0, 0] * s0 + U[0, 1] * s1
        out[:, :, 1, :] = U[1, 0] * s0 + U[1, 1] * s1
        return out.reshape(-1, DIM)

    def rx(t):
        c = np.cos(t / 2)
        s = -1j * np.sin(t / 2)
        return np.array([[c, s], [s, c]], dtype=np.complex128)

    def rz(t):
        return np.array(
            [[np.exp(-0.5j * t), 0], [0, np.exp(0.5j * t)]], dtype=np.complex128
        )

    d = 0
    for i in range(N_QUBITS):
        M = apply_1q(M, rx(w[d, i, 0]), i)
        M = apply_1q(M, rz(w[d, i, 1]), i)
        M = apply_1q(M, rx(w[d, i, 2]), i)
    j = 0
    idx = np.arange(DIM)
    for i in range(N_QUBITS):
        nj = (j + (N_QUBITS - 3)) % N_QUBITS
        hit = (
            (idx >> (N_QUBITS - 1 - j)) & (idx >> (N_QUBITS - 1 - nj)) & 1
        ).astype(bool)
        phase = np.where(hit, np.exp(1j * w[d, i, 3]), 1.0).astype(np.complex128)
        M = M * phase[None, :]
        M = apply_1q(M, rx(w[d, i, 4]), nj)
        j = nj
    return M


def _signs_aug():
    """(DIM, NAUG) fp32: PauliZ eigenvalue columns plus a ones column."""
    idx = np.arange(DIM)
    bits = (idx[:, None] >> (N_QUBITS - 1 - np.arange(N_QUBITS))[None, :]) & 1
    s = (1.0 - 2.0 * bits).astype(np.float32)
    return np.concatenate([s, np.ones((DIM, 1), np.float32)], axis=1)


def _pack_k_major(a):
    """(DIM, C) -> (128, KT, C): slab[p, t, c] = a[t*128 + p, c]."""
    c = a.shape[1]
    return np.ascontiguousarray(a.reshape(KT, 128, c).transpose(1, 0, 2))


def _pack_m(m):
    """(DIM, DIM) [k, j] -> (JT, 128, KT, 128): [jt][p, kt, j]."""
    a = m.reshape(KT, 128, JT, 128).transpose(2, 1, 0, 3)
    return np.ascontiguousarray(a)


# --------------------------------------------------------------- bass kernel

_CACHED_NC = None


def _build_bass():
    from contextlib import ExitStack

    nc = bass.Bass("TRN2")
    xt_d = nc.dram_tensor("xt", (128, KT, BS), MM_DTYPE,
                          kind="ExternalInput")
    mre_d = nc.dram_tensor("m_re", (JT, 128, KT, 128), MM_DTYPE,
                           kind="ExternalInput")
    mim_d = nc.dram_tensor("m_im", (JT, 128, KT, 128), MM_DTYPE,
                           kind="ExternalInput")
    sgn_d = nc.dram_tensor("sgn", (128, KT, NAUG), MM_DTYPE,
                           kind="ExternalInput")
    ident_d = nc.dram_tensor("ident", (NAUG, NAUG), mybir.dt.float32,
                             kind="ExternalInput")
    out_d = nc.dram_tensor("out", (BS, N_QUBITS), mybir.dt.float32,
                           kind="ExternalOutput")

    with ExitStack() as es:
        tc = es.enter_context(tile.TileContext(nc))
        singles = es.enter_context(tc.tile_pool(name="singles", bufs=1))
        # one buffer per jt: kills every tmp-slot WAR/WAW dep
        tmps = es.enter_context(tc.tile_pool(name="tmps", bufs=JT))
        psum = es.enter_context(tc.tile_pool(name="psum", bufs=2, space="PSUM"))
        psum_o = es.enter_context(
            tc.tile_pool(name="psum_o", bufs=2, space="PSUM"))
        psum_w = es.enter_context(
            tc.tile_pool(name="psum_w", bufs=1, space="PSUM"))

        BF = mybir.dt.bfloat16

        def pe_gate(ap):
            """Real PE-engine instruction (1-column bf16 ldweights) whose sole
            purpose is to carry one sync wait for `ap`'s producer; following
            matmuls then inherit the observed clock."""
            return nc.tensor.ldweights(weights=ap.bitcast(BF))

        def after(inst, gates):
            # sync=True same-engine edge: no semaphore, joins vector clocks,
            # pins scheduling order.
            for g in gates:
                add_dep_helper(inst.ins, g.ins, True, "order-after-gate")

        # ---- PE warmup ----
        # The PE clock is HAM-gated at 1.2 GHz until ~3.4us of sustained
        # activity.  The PE would otherwise idle during the xt/slab loads,
        # so burn that window on dummy matmuls over a zeroed tile: the real
        # matmuls then start at the full 2.4 GHz.
        zero_sb = singles.tile([128, BS], mybir.dt.float32, tag="zero")
        nc.vector.memset(zero_sb, 0)
        warm_ps = psum_w.tile([128, BS], mybir.dt.float32, tag="warm")
        N_WARM = 6
        for i in range(N_WARM):
            nc.tensor.matmul(
                warm_ps,
                lhsT=zero_sb[:, 0:128].bitcast(MM_DTYPE),
                rhs=zero_sb[:].bitcast(MM_DTYPE),
                start=(i == 0),
                stop=(i == N_WARM - 1),
            )

        # ---- loads ----
        # xt split in four chunks interleaved with the jt=0 slabs, so the
        # first matmul group starts as soon as possible and streams.
        xt_sb = singles.tile([128, KT, BS], MM_DTYPE, tag="xt")
        mre_sb = singles.tile([128, JT, KT, 128], MM_DTYPE, tag="mre")
        mim_sb = singles.tile([128, JT, KT, 128], MM_DTYPE, tag="mim")
        sgn_sb = singles.tile([128, KT, NAUG], MM_DTYPE, tag="sgn")
        ident_sb = singles.tile([NAUG, NAUG], mybir.dt.float32, tag="ident")
        XC = KT // 4
        nc.sync.dma_start(out=xt_sb[:, 0:XC, :], in_=xt_d[:, 0:XC, :])
        nc.sync.dma_start(out=mre_sb[:, 0], in_=mre_d[0])
        nc.sync.dma_start(out=xt_sb[:, XC:2 * XC, :], in_=xt_d[:, XC:2 * XC, :])
        nc.sync.dma_start(out=mim_sb[:, 0], in_=mim_d[0])
        nc.sync.dma_start(out=xt_sb[:, 2 * XC:3 * XC, :],
                          in_=xt_d[:, 2 * XC:3 * XC, :])
        nc.sync.dma_start(out=sgn_sb, in_=sgn_d[:])
        nc.sync.dma_start(out=xt_sb[:, 3 * XC:, :], in_=xt_d[:, 3 * XC:, :])
        nc.sync.dma_start(out=ident_sb, in_=ident_d[:])
        for jt in range(1, JT):
            nc.sync.dma_start(out=mre_sb[:, jt], in_=mre_d[jt])
            nc.sync.dma_start(out=mim_sb[:, jt], in_=mim_d[jt])

        probs_sb = singles.tile([128, JT, BS], MM_DTYPE, tag="probs")
        out_all = singles.tile([128, BS // 128, N_QUBITS], mybir.dt.float32,
                               tag="out_all")

        # PE observes the four xt chunks (1 wait each, none downstream)
        g_xt = [pe_gate(xt_sb[:, c * XC, 0:1]) for c in range(4)]

        sq_tiles = {}
        outT_ps = psum_w.tile([NAUG, BS], mybir.dt.float32, tag="outT")
        g_sgn = pe_gate(sgn_sb[:, 0, 0:1])

        def mm_group(part, m_sb, jt, ps):
            gates = [pe_gate(m_sb[:, jt, 0, 0:1]), g_xt[0]]
            if jt >= 2:
                # psum slot last read by the square 2 rounds ago: observing
                # that square's output imports the needed ACT tick
                gates.append(pe_gate(sq_tiles[(part, jt - 2)][:, 0:1]))
            for kt in range(KT):
                mm = nc.tensor.matmul(
                    ps,
                    lhsT=m_sb[:, jt, kt, :].bitcast(MM_DTYPE),
                    rhs=xt_sb[:, kt, :].bitcast(MM_DTYPE),
                    start=(kt == 0),
                    stop=(kt == KT - 1),
                )
                if kt == 0:
                    after(mm, gates)
                elif kt % XC == 0:
                    after(mm, [g_xt[kt // XC]])

        for jt in range(JT):
            ps_re = psum.tile([128, BS], mybir.dt.float32, tag="ps_re")
            mm_group("re", mre_sb, jt, ps_re)
            ps_im = psum.tile([128, BS], mybir.dt.float32, tag="ps_im")
            mm_group("im", mim_sb, jt, ps_im)

            # squares on ACT (sole PSUM reader), sum on DVE (sole probs writer)
            sq_re = tmps.tile([128, BS], mybir.dt.float32, tag="sq_re")
            sq_im = tmps.tile([128, BS], mybir.dt.float32, tag="sq_im")
            nc.scalar.activation(
                out=sq_re, in_=ps_re,
                func=mybir.ActivationFunctionType.Square,
            )
            nc.scalar.activation(
                out=sq_im, in_=ps_im,
                func=mybir.ActivationFunctionType.Square,
            )
            sq_tiles[("re", jt)] = sq_re
            sq_tiles[("im", jt)] = sq_im
            nc.vector.tensor_add(probs_sb[:, jt, :], sq_re, sq_im)

            # fold this jt's probs into the signs contraction right away:
            # signs stationary (11-col weight load is ~free), probs moving.
            mo = nc.tensor.matmul(
                outT_ps,
                lhsT=sgn_sb[:, jt, :],
                rhs=probs_sb[:, jt, :],
                start=(jt == 0),
                stop=(jt == JT - 1),
                skip_group_check=True,
            )
            if jt == 0:
                after(mo, [g_sgn])

        # ---- transpose [11, BS] -> 4x [128, 11], then normalize ----
        outT_sb = singles.tile([NAUG, BS], mybir.dt.float32, tag="outT")
        nc.vector.tensor_copy(out=outT_sb, in_=outT_ps)
        g_outT = pe_gate(outT_sb[:, 0:1])
        g_ident = pe_gate(ident_sb[:, 0:1])
        for bc in range(BS // 128):
            gates_o = [g_outT, g_ident]
            if bc >= 2:
                # ps_o slot last read by the DVE mul two rounds ago
                gates_o.append(pe_gate(out_all[:, bc - 2, 0:1]))
            ps_o = psum_o.tile([128, NAUG], mybir.dt.float32, tag="ps_o")
            mm = nc.tensor.matmul(
                ps_o,
                lhsT=outT_sb[:, bc * 128:(bc + 1) * 128],
                rhs=ident_sb[:],
                start=True,
                stop=True,
            )
            after(mm, gates_o)
            recip = tmps.tile([128, 1], mybir.dt.float32, tag="recip")
            nc.vector.reciprocal(out=recip, in_=ps_o[:, N_QUBITS:N_QUBITS + 1])
            nc.vector.tensor_scalar_mul(out_all[:, bc, :], ps_o[:, 0:N_QUBITS],
                                        recip)

        # single SWDGE store: separate semaphore lanes from the HWDGE loads,
        # so the only wait is the DVE producer
        nc.gpsimd.dma_start(
            out=out_d.rearrange("(c p) q -> p c q", p=128), in_=out_all)

    return nc


def _get_nc():
    global _CACHED_NC
    if _CACHED_NC is None:
        _CACHED_NC = _build_bass()
    return _CACHED_NC


# ----------------------------------------------------------------- entrypoint

def kernel(x, weights, weights_1, weights_2, _trace=False):
    x = np.asarray(x, dtype=np.float32)
    xf = x.reshape(B, DIM)

    M = _build_circuit_matrix(weights)
    mre_pack = _pack_m(M.real.astype(np.float32))
    mim_pack = _pack_m(M.imag.astype(np.float32))
    sgn_pack = _pack_k_major(_signs_aug())

    in_maps = []
    for c in range(N_CORES):
        shard = xf[c * BS:(c + 1) * BS]              # (BS, DIM)
        xt = np.ascontiguousarray(shard.T)           # (DIM, BS)
        xt_pack = _pack_k_major(xt)                  # (128, KT, BS)
        in_maps.append({
            "xt": xt_pack,
            "m_re": mre_pack,
            "m_im": mim_pack,
            "sgn": sgn_pack,
            "ident": np.eye(NAUG, dtype=np.float32),
        })

    nc = _get_nc()
    res = run_bass_kernel_spmd(nc, in_maps, core_ids=list(range(N_CORES)),
                               trace=_trace)
    out = np.concatenate([r["out"] for r in res.results], axis=0)
    if _trace:
        kernel.last_exec_time_ns = res.exec_time_ns
        kernel.last_results = res
    return out.astype(np.float32)


# revision 22
# speedup vs baseline: 1.0368x; 1.0368x over previous
"""Trainium2 Bass kernel for the CCQC quantum-circuit classifier.

The whole circuit (one layer: RX/RZ/RX per qubit, then CPhase+RX ring) is a
fixed linear operator on the 1024-dim state vector.  On the host we fold all
40 gates into a single 1024x1024 complex matrix M (cheap: ~1s of numpy on
2x1024x1024 floats), so that for a batch row xf:

    state_final = xf @ M            (xf real, M complex)
    probs       = |state_final|^2
    out         = (probs @ signsT) / ||xf||^2

The division uses unitarity: sum_j probs[j] = ||xf||^2, so an extra ones
column appended to signsT yields the normalizer for free.

Device work per core (batch 512 of 4096):
    RE^T = M_re^T x^T, IM^T = M_im^T x^T   (TensorE, K=1024 contraction)
    probsT = RE^2 + IM^2                    (ScalarE square + VectorE add)
    outT   = [signs|1]^T probsT             (TensorE, contraction over 1024)
    out    = outT[:,0:10] * recip(outT[:,10])

The walrus build in this container allows AT MOST ONE sync-wait per
instruction.  All cross-engine dependencies are therefore funneled through
explicit single-wait NOP "gates" (nofuse=True) wired with add_dep_helper;
each real instruction is left with at most one unobserved semaphore.
"""

import numpy as np

import concourse.bass as bass
import concourse.tile as tile
from concourse import mybir
from concourse.bass_utils import run_bass_kernel_spmd
from concourse.tile_rust import add_dep_helper
from concourse.vector_clock import ScopedClock, VectorClock

# The walrus build here accepts at most ONE sync wait per instruction, but
# Tile's kernel-tail emits a single Drain waiting on every proc's semaphore.
# Split that into a chain of single-wait pre-drains (one proc each); the
# final stock drain then finds everything already observed and gets no waits.
from concourse.tile_sem_assignment import tick_to_sem


def _split_drain_and_barrier(self, tick_clock, wait_clock):
    ticks = eval(repr(tick_clock.global_clock)
                 .replace("VectorClock(", "").rstrip(")"))
    allocated = dict(wait_clock.sems.allocated())
    for p, t in enumerate(ticks):
        if t > 0 and p in allocated:
            self.nc.sync.wait_ge(allocated[p], tick_to_sem(t, p))
    self.nc.sync.drain()
    self.nc.all_engine_barrier()
    popped = self.nc._tile_sem_poison_stack.pop()
    assert popped is self._sem_poison
    self.nc.clear_and_free_semaphores(list(self.sems.allocated().values()))
    self.nc.all_engine_barrier()


tile.TileContext._drain_and_barrier = _split_drain_and_barrier

N_CORES = 8
N_QUBITS = 10
DIM = 1 << N_QUBITS          # 1024
B = 4096
BS = B // N_CORES            # 512 rows per core
KT = DIM // 128              # 8 contraction tiles
JT = DIM // 128              # 8 output-column tiles
NAUG = N_QUBITS + 1          # signs columns + ones column

# matmul dtype for the two big (512x1024x1024) products:
#   float32  = exact, 4 cycles/row on the PE
#   float32r = TF32-like reduced precision, 1 cycle/row (4x faster)
MM_DTYPE = mybir.dt.float32r


# ----------------------------------------------------------------- host math

def _build_circuit_matrix(weights):
    """M (DIM, DIM) complex128 with final_state_row = xf_row @ M."""
    w = np.asarray(weights, dtype=np.float64)
    M = np.eye(DIM, dtype=np.complex128)

    def apply_1q(state, U, wire):
        left = 1 << wire
        right = 1 << (N_QUBITS - 1 - wire)
        s = state.reshape(-1, left, 2, right)
        s0 = s[:, :, 0, :]
        s1 = s[:, :, 1, :]
        out = np.empty_like(s)
        out[:, :, 0, :] = U[0, 0] * s0 + U[0, 1] * s1
        out[:, :, 1, :] = U[1, 0] * s0 + U[1, 1] * s1
        return out.reshape(-1, DIM)

    def rx(t):
        c = np.cos(t / 2)
        s = -1j * np.sin(t / 2)
        return np.array([[c, s], [s, c]], dtype=np.complex128)

    def rz(t):
        return np.array(
            [[np.exp(-0.5j * t), 0], [0, np.exp(0.5j * t)]], dtype=np.complex128
        )

    d = 0
    for i in range(N_QUBITS):
        M = apply_1q(M, rx(w[d, i, 0]), i)
        M = apply_1q(M, rz(w[d, i, 1]), i)
        M = apply_1q(M, rx(w[d, i, 2]), i)
    j = 0
    idx = np.arange(DIM)
    for i in range(N_QUBITS):
        nj = (j + (N_QUBITS - 3)) % N_QUBITS
        hit = (
            (idx >> (N_QUBITS - 1 - j)) & (idx >> (N_QUBITS - 1 - nj)) & 1
        ).astype(bool)
        phase = np.where(hit, np.exp(1j * w[d, i, 3]), 1.0).astype(np.complex128)
        M = M * phase[None, :]
        M = apply_1q(M, rx(w[d, i, 4]), nj)
        j = nj
    return M


def _signs_aug():
    """(DIM, NAUG) fp32: PauliZ eigenvalue columns plus a ones column."""
    idx = np.arange(DIM)
    bits = (idx[:, None] >> (N_QUBITS - 1 - np.arange(N_QUBITS))[None, :]) & 1
    s = (1.0 - 2.0 * bits).astype(np.float32)
    return np.concatenate([s, np.ones((DIM, 1), np.float32)], axis=1)


def _pack_k_major(a):
    """(DIM, C) -> (128, KT, C): slab[p, t, c] = a[t*128 + p, c]."""
    c = a.shape[1]
    return np.ascontiguousarray(a.reshape(KT, 128, c).transpose(1, 0, 2))


def _pack_m(m):
    """(DIM, DIM) [k, j] -> (JT, 128, KT, 128): [jt][p, kt, j]."""
    a = m.reshape(KT, 128, JT, 128).transpose(2, 1, 0, 3)
    return np.ascontiguousarray(a)


# --------------------------------------------------------------- bass kernel

_CACHED_NC = None


def _build_bass():
    from contextlib import ExitStack

    nc = bass.Bass("TRN2")
    xt_d = nc.dram_tensor("xt", (128, KT, BS), MM_DTYPE,
                          kind="ExternalInput")
    mre_d = nc.dram_tensor("m_re", (JT, 128, KT, 128), MM_DTYPE,
                           kind="ExternalInput")
    mim_d = nc.dram_tensor("m_im", (JT, 128, KT, 128), MM_DTYPE,
                           kind="ExternalInput")
    sgn_d = nc.dram_tensor("sgn", (128, KT, NAUG), MM_DTYPE,
                           kind="ExternalInput")
    ident_d = nc.dram_tensor("ident", (NAUG, NAUG), mybir.dt.float32,
                             kind="ExternalInput")
    out_d = nc.dram_tensor("out", (BS, N_QUBITS), mybir.dt.float32,
                           kind="ExternalOutput")

    with ExitStack() as es:
        tc = es.enter_context(tile.TileContext(nc))
        singles = es.enter_context(tc.tile_pool(name="singles", bufs=1))
        # one buffer per jt: kills every tmp-slot WAR/WAW dep
        tmps = es.enter_context(tc.tile_pool(name="tmps", bufs=JT))
        psum = es.enter_context(tc.tile_pool(name="psum", bufs=2, space="PSUM"))
        psum_o = es.enter_context(
            tc.tile_pool(name="psum_o", bufs=2, space="PSUM"))
        psum_w = es.enter_context(
            tc.tile_pool(name="psum_w", bufs=1, space="PSUM"))

        BF = mybir.dt.bfloat16

        def pe_gate(ap):
            """Real PE-engine instruction (1-column bf16 ldweights) whose sole
            purpose is to carry one sync wait for `ap`'s producer; following
            matmuls then inherit the observed clock."""
            return nc.tensor.ldweights(weights=ap.bitcast(BF))

        def after(inst, gates):
            # sync=True same-engine edge: no semaphore, joins vector clocks,
            # pins scheduling order.
            for g in gates:
                add_dep_helper(inst.ins, g.ins, True, "order-after-gate")

        # ---- PE warmup ----
        # The PE clock is HAM-gated at 1.2 GHz until ~3.4us of sustained
        # activity.  The PE would otherwise idle during the xt/slab loads,
        # so burn that window on dummy matmuls over a zeroed tile: the real
        # matmuls then start at the full 2.4 GHz.
        zero_sb = singles.tile([128, BS], mybir.dt.float32, tag="zero")
        nc.vector.memset(zero_sb, 0)
        warm_ps = psum_w.tile([128, BS], mybir.dt.float32, tag="warm")
        N_WARM = 7
        for i in range(N_WARM):
            nc.tensor.matmul(
                warm_ps,
                lhsT=zero_sb[:, 0:128].bitcast(MM_DTYPE),
                rhs=zero_sb[:].bitcast(MM_DTYPE),
                start=(i == 0),
                stop=(i == N_WARM - 1),
            )

        # ---- loads ----
        # xt split in four chunks interleaved with the jt=0 slabs, so the
        # first matmul group starts as soon as possible and streams.
        xt_sb = singles.tile([128, KT, BS], MM_DTYPE, tag="xt")
        mre_sb = singles.tile([128, JT, KT, 128], MM_DTYPE, tag="mre")
        mim_sb = singles.tile([128, JT, KT, 128], MM_DTYPE, tag="mim")
        sgn_sb = singles.tile([128, KT, NAUG], MM_DTYPE, tag="sgn")
        ident_sb = singles.tile([NAUG, NAUG], mybir.dt.float32, tag="ident")
        XC = KT // 2
        nc.sync.dma_start(out=xt_sb[:, 0:XC, :], in_=xt_d[:, 0:XC, :])
        nc.sync.dma_start(out=mre_sb[:, 0], in_=mre_d[0])
        nc.sync.dma_start(out=mim_sb[:, 0], in_=mim_d[0])
        nc.sync.dma_start(out=xt_sb[:, XC:, :], in_=xt_d[:, XC:, :])
        nc.sync.dma_start(out=sgn_sb, in_=sgn_d[:])
        nc.sync.dma_start(out=ident_sb, in_=ident_d[:])
        for jt in range(1, JT):
            nc.sync.dma_start(out=mre_sb[:, jt], in_=mre_d[jt])
            nc.sync.dma_start(out=mim_sb[:, jt], in_=mim_d[jt])

        probs_sb = singles.tile([128, JT, BS], MM_DTYPE, tag="probs")
        out_all = singles.tile([128, BS // 128, N_QUBITS], mybir.dt.float32,
                               tag="out_all")

        # PE observes the two xt halves (1 wait each, none downstream)
        g_xt = [pe_gate(xt_sb[:, c * XC, 0:1]) for c in range(2)]

        sq_tiles = {}
        outT_ps = psum_w.tile([NAUG, BS], mybir.dt.float32, tag="outT")
        g_sgn = pe_gate(sgn_sb[:, 0, 0:1])

        def mm_group(part, m_sb, jt, ps):
            gates = [pe_gate(m_sb[:, jt, 0, 0:1]), g_xt[0]]
            if jt >= 2:
                # psum slot last read by the square 2 rounds ago: observing
                # that square's output imports the needed ACT tick
                gates.append(pe_gate(sq_tiles[(part, jt - 2)][:, 0:1]))
            for kt in range(KT):
                mm = nc.tensor.matmul(
                    ps,
                    lhsT=m_sb[:, jt, kt, :].bitcast(MM_DTYPE),
                    rhs=xt_sb[:, kt, :].bitcast(MM_DTYPE),
                    start=(kt == 0),
                    stop=(kt == KT - 1),
                )
                if kt == 0:
                    after(mm, gates)
                elif kt % XC == 0:
                    after(mm, [g_xt[kt // XC]])

        for jt in range(JT):
            ps_re = psum.tile([128, BS], mybir.dt.float32, tag="ps_re")
            mm_group("re", mre_sb, jt, ps_re)
            ps_im = psum.tile([128, BS], mybir.dt.float32, tag="ps_im")
            mm_group("im", mim_sb, jt, ps_im)

            # squares on ACT (sole PSUM reader), sum on DVE (sole probs writer)
            sq_re = tmps.tile([128, BS], mybir.dt.float32, tag="sq_re")
            sq_im = tmps.tile([128, BS], mybir.dt.float32, tag="sq_im")
            nc.scalar.activation(
                out=sq_re, in_=ps_re,
                func=mybir.ActivationFunctionType.Square,
            )
            nc.scalar.activation(
                out=sq_im, in_=ps_im,
                func=mybir.ActivationFunctionType.Square,
            )
            sq_tiles[("re", jt)] = sq_re
            sq_tiles[("im", jt)] = sq_im
            nc.vector.tensor_add(probs_sb[:, jt, :], sq_re, sq_im)

            # fold this jt's probs into the signs contraction right away:
            # signs stationary (11-col weight load is ~free), probs moving.
            mo = nc.tensor.matmul(
                outT_ps,
                lhsT=sgn_sb[:, jt, :],
                rhs=probs_sb[:, jt, :],
                start=(jt == 0),
                stop=(jt == JT - 1),
                skip_group_check=True,
            )
            if jt == 0:
                after(mo, [g_sgn])

        # ---- transpose [11, BS] -> 4x [128, 11], then normalize ----
        outT_sb = singles.tile([NAUG, BS], mybir.dt.float32, tag="outT")
        nc.vector.tensor_copy(out=outT_sb, in_=outT_ps)
        g_outT = pe_gate(outT_sb[:, 0:1])
        g_ident = pe_gate(ident_sb[:, 0:1])
        for bc in range(BS // 128):
            gates_o = [g_outT, g_ident]
            if bc >= 2:
                # ps_o slot last read by the DVE mul two rounds ago
                gates_o.append(pe_gate(out_all[:, bc - 2, 0:1]))
            ps_o = psum_o.tile([128, NAUG], mybir.dt.float32, tag="ps_o")
            mm = nc.tensor.matmul(
                ps_o,
                lhsT=outT_sb[:, bc * 128:(bc + 1) * 128],
                rhs=ident_sb[:],
                start=True,
                stop=True,
            )
            after(mm, gates_o)
            recip = tmps.tile([128, 1], mybir.dt.float32, tag="recip")
            nc.vector.reciprocal(out=recip, in_=ps_o[:, N_QUBITS:N_QUBITS + 1])
            nc.vector.tensor_scalar_mul(out_all[:, bc, :], ps_o[:, 0:N_QUBITS],
                                        recip)

        # single SWDGE store: separate semaphore lanes from the HWDGE loads,
        # so the only wait is the DVE producer
        nc.gpsimd.dma_start(
            out=out_d.rearrange("(c p) q -> p c q", p=128), in_=out_all)

    return nc


def _get_nc():
    global _CACHED_NC
    if _CACHED_NC is None:
        _CACHED_NC = _build_bass()
    return _CACHED_NC


# ----------------------------------------------------------------- entrypoint

def kernel(x, weights, weights_1, weights_2, _trace=False):
    x = np.asarray(x, dtype=np.float32)
    xf = x.reshape(B, DIM)

    M = _build_circuit_matrix(weights)
    mre_pack = _pack_m(M.real.astype(np.float32))
    mim_pack = _pack_m(M.imag.astype(np.float32))
    sgn_pack = _pack_k_major(_signs_aug())

    in_maps = []
    for c in range(N_CORES):
        shard = xf[c * BS:(c + 1) * BS]              # (BS, DIM)
        xt = np.ascontiguousarray(shard.T)           # (DIM, BS)
        xt_pack = _pack_k_major(xt)                  # (128, KT, BS)
        in_maps.append({
            "xt": xt_pack,
            "m_re": mre_pack,
            "m_im": mim_pack,
            "sgn": sgn_pack,
            "ident": np.eye(NAUG, dtype=np.float32),
        })

    nc = _get_nc()
    res = run_bass_kernel_spmd(nc, in_maps, core_ids=list(range(N_CORES)),
                               trace=_trace)
    out = np.concatenate([r["out"] for r in res.results], axis=0)
    if _trace:
        kernel.last_exec_time_ns = res.exec_time_ns
        kernel.last_results = res
    return out.astype(np.float32)


# revision 25
# speedup vs baseline: 1.0675x; 1.0297x over previous
"""Trainium2 Bass kernel for the CCQC quantum-circuit classifier.

The whole circuit (one layer: RX/RZ/RX per qubit, then CPhase+RX ring) is a
fixed linear operator on the 1024-dim state vector.  On the host we fold all
40 gates into a single 1024x1024 complex matrix M (cheap: ~1s of numpy on
2x1024x1024 floats), so that for a batch row xf:

    state_final = xf @ M            (xf real, M complex)
    probs       = |state_final|^2
    out         = (probs @ signsT) / ||xf||^2

The division uses unitarity: sum_j probs[j] = ||xf||^2, so an extra ones
column appended to signsT yields the normalizer for free.

Device work per core (batch 512 of 4096):
    RE^T = M_re^T x^T, IM^T = M_im^T x^T   (TensorE, K=1024 contraction)
    probsT = RE^2 + IM^2                    (ScalarE square + VectorE add)
    outT   = [signs|1]^T probsT             (TensorE, contraction over 1024)
    out    = outT[:,0:10] * recip(outT[:,10])

The walrus build in this container allows AT MOST ONE sync-wait per
instruction.  All cross-engine dependencies are therefore funneled through
explicit single-wait NOP "gates" (nofuse=True) wired with add_dep_helper;
each real instruction is left with at most one unobserved semaphore.
"""

import numpy as np

import concourse.bass as bass
import concourse.tile as tile
from concourse import mybir
from concourse.bass_utils import run_bass_kernel_spmd
from concourse.tile_rust import add_dep_helper
from concourse.vector_clock import ScopedClock, VectorClock

# The walrus build here accepts at most ONE sync wait per instruction, but
# Tile's kernel-tail emits a single Drain waiting on every proc's semaphore.
# Split that into a chain of single-wait pre-drains (one proc each); the
# final stock drain then finds everything already observed and gets no waits.
from concourse.tile_sem_assignment import tick_to_sem


def _split_drain_and_barrier(self, tick_clock, wait_clock):
    ticks = eval(repr(tick_clock.global_clock)
                 .replace("VectorClock(", "").rstrip(")"))
    allocated = dict(wait_clock.sems.allocated())
    for p, t in enumerate(ticks):
        if t > 0 and p in allocated:
            self.nc.sync.wait_ge(allocated[p], tick_to_sem(t, p))
    self.nc.sync.drain()
    self.nc.all_engine_barrier()
    popped = self.nc._tile_sem_poison_stack.pop()
    assert popped is self._sem_poison
    self.nc.clear_and_free_semaphores(list(self.sems.allocated().values()))
    self.nc.all_engine_barrier()


tile.TileContext._drain_and_barrier = _split_drain_and_barrier

N_CORES = 8
N_QUBITS = 10
DIM = 1 << N_QUBITS          # 1024
B = 4096
BS = B // N_CORES            # 512 rows per core
KT = DIM // 128              # 8 contraction tiles
JT = DIM // 128              # 8 output-column tiles
NAUG = N_QUBITS + 1          # signs columns + ones column

# matmul dtype for the two big (512x1024x1024) products:
#   float32  = exact, 4 cycles/row on the PE
#   float32r = TF32-like reduced precision, 1 cycle/row (4x faster)
MM_DTYPE = mybir.dt.float32r


# ----------------------------------------------------------------- host math

def _build_circuit_matrix(weights):
    """M (DIM, DIM) complex128 with final_state_row = xf_row @ M."""
    w = np.asarray(weights, dtype=np.float64)
    M = np.eye(DIM, dtype=np.complex128)

    def apply_1q(state, U, wire):
        left = 1 << wire
        right = 1 << (N_QUBITS - 1 - wire)
        s = state.reshape(-1, left, 2, right)
        s0 = s[:, :, 0, :]
        s1 = s[:, :, 1, :]
        out = np.empty_like(s)
        out[:, :, 0, :] = U[0, 0] * s0 + U[0, 1] * s1
        out[:, :, 1, :] = U[1, 0] * s0 + U[1, 1] * s1
        return out.reshape(-1, DIM)

    def rx(t):
        c = np.cos(t / 2)
        s = -1j * np.sin(t / 2)
        return np.array([[c, s], [s, c]], dtype=np.complex128)

    def rz(t):
        return np.array(
            [[np.exp(-0.5j * t), 0], [0, np.exp(0.5j * t)]], dtype=np.complex128
        )

    d = 0
    for i in range(N_QUBITS):
        M = apply_1q(M, rx(w[d, i, 0]), i)
        M = apply_1q(M, rz(w[d, i, 1]), i)
        M = apply_1q(M, rx(w[d, i, 2]), i)
    j = 0
    idx = np.arange(DIM)
    for i in range(N_QUBITS):
        nj = (j + (N_QUBITS - 3)) % N_QUBITS
        hit = (
            (idx >> (N_QUBITS - 1 - j)) & (idx >> (N_QUBITS - 1 - nj)) & 1
        ).astype(bool)
        phase = np.where(hit, np.exp(1j * w[d, i, 3]), 1.0).astype(np.complex128)
        M = M * phase[None, :]
        M = apply_1q(M, rx(w[d, i, 4]), nj)
        j = nj
    return M


def _signs_aug():
    """(DIM, NAUG) fp32: PauliZ eigenvalue columns plus a ones column."""
    idx = np.arange(DIM)
    bits = (idx[:, None] >> (N_QUBITS - 1 - np.arange(N_QUBITS))[None, :]) & 1
    s = (1.0 - 2.0 * bits).astype(np.float32)
    return np.concatenate([s, np.ones((DIM, 1), np.float32)], axis=1)


def _pack_k_major(a):
    """(DIM, C) -> (128, KT, C): slab[p, t, c] = a[t*128 + p, c]."""
    c = a.shape[1]
    return np.ascontiguousarray(a.reshape(KT, 128, c).transpose(1, 0, 2))


def _pack_m(m):
    """(DIM, DIM) [k, j] -> (JT, 128, KT, 128): [jt][p, kt, j]."""
    a = m.reshape(KT, 128, JT, 128).transpose(2, 1, 0, 3)
    return np.ascontiguousarray(a)


# --------------------------------------------------------------- bass kernel

_CACHED_NC = None


def _build_bass():
    from contextlib import ExitStack

    nc = bass.Bass("TRN2")
    xt_d = nc.dram_tensor("xt", (128, KT, BS), MM_DTYPE,
                          kind="ExternalInput")
    mre_d = nc.dram_tensor("m_re", (JT, 128, KT, 128), MM_DTYPE,
                           kind="ExternalInput")
    mim_d = nc.dram_tensor("m_im", (JT, 128, KT, 128), MM_DTYPE,
                           kind="ExternalInput")
    sgn_d = nc.dram_tensor("sgn", (128, KT, NAUG), MM_DTYPE,
                           kind="ExternalInput")
    ident_d = nc.dram_tensor("ident", (NAUG, NAUG), mybir.dt.float32,
                             kind="ExternalInput")
    out_d = nc.dram_tensor("out", (BS, N_QUBITS), mybir.dt.float32,
                           kind="ExternalOutput")

    with ExitStack() as es:
        tc = es.enter_context(tile.TileContext(nc))
        singles = es.enter_context(tc.tile_pool(name="singles", bufs=1))
        # one buffer per jt: kills every tmp-slot WAR/WAW dep
        tmps = es.enter_context(tc.tile_pool(name="tmps", bufs=JT))
        psum = es.enter_context(tc.tile_pool(name="psum", bufs=2, space="PSUM"))
        psum_o = es.enter_context(
            tc.tile_pool(name="psum_o", bufs=2, space="PSUM"))
        psum_w = es.enter_context(
            tc.tile_pool(name="psum_w", bufs=1, space="PSUM"))

        BF = mybir.dt.bfloat16

        def pe_gate(ap):
            """Real PE-engine instruction (1-column bf16 ldweights) whose sole
            purpose is to carry one sync wait for `ap`'s producer; following
            matmuls then inherit the observed clock."""
            return nc.tensor.ldweights(weights=ap.bitcast(BF))

        def after(inst, gates):
            # sync=True same-engine edge: no semaphore, joins vector clocks,
            # pins scheduling order.
            for g in gates:
                add_dep_helper(inst.ins, g.ins, True, "order-after-gate")

        # ---- PE warmup ----
        # The PE clock is HAM-gated at 1.2 GHz until ~3.4us of sustained
        # activity.  The PE would otherwise idle during the xt/slab loads,
        # so burn that window on dummy matmuls over a zeroed tile: the real
        # matmuls then start at the full 2.4 GHz.
        zero_sb = singles.tile([128, BS], mybir.dt.float32, tag="zero")
        nc.vector.memset(zero_sb, 0)
        warm_ps = psum_w.tile([128, BS], mybir.dt.float32, tag="warm")
        N_WARM = 7
        for i in range(N_WARM):
            nc.tensor.matmul(
                warm_ps,
                lhsT=zero_sb[:, 0:128].bitcast(MM_DTYPE),
                rhs=zero_sb[:].bitcast(MM_DTYPE),
                start=(i == 0),
                stop=(i == N_WARM - 1),
            )

        # ---- loads ----
        # xt split in four chunks interleaved with the jt=0 slabs, so the
        # first matmul group starts as soon as possible and streams.
        xt_sb = singles.tile([128, KT, BS], MM_DTYPE, tag="xt")
        mre_sb = singles.tile([128, JT, KT, 128], MM_DTYPE, tag="mre")
        mim_sb = singles.tile([128, JT, KT, 128], MM_DTYPE, tag="mim")
        sgn_sb = singles.tile([128, KT, NAUG], MM_DTYPE, tag="sgn")
        ident_sb = singles.tile([NAUG, NAUG], mybir.dt.float32, tag="ident")
        XC = KT // 2
        nc.sync.dma_start(out=xt_sb[:, 0:XC, :], in_=xt_d[:, 0:XC, :])
        nc.sync.dma_start(out=mre_sb[:, 0], in_=mre_d[0])
        nc.sync.dma_start(out=mim_sb[:, 0], in_=mim_d[0])
        nc.sync.dma_start(out=xt_sb[:, XC:, :], in_=xt_d[:, XC:, :])
        nc.sync.dma_start(out=sgn_sb, in_=sgn_d[:])
        nc.sync.dma_start(out=ident_sb, in_=ident_d[:])
        for jt in range(1, JT):
            nc.sync.dma_start(out=mre_sb[:, jt], in_=mre_d[jt])
            nc.sync.dma_start(out=mim_sb[:, jt], in_=mim_d[jt])

        probs_sb = singles.tile([128, JT, BS], MM_DTYPE, tag="probs")
        out_all = singles.tile([128, BS // 128, N_QUBITS], mybir.dt.float32,
                               tag="out_all")

        # PE observes the two xt halves (1 wait each, none downstream)
        g_xt = [pe_gate(xt_sb[:, c * XC, 0:1]) for c in range(2)]

        sq_tiles = {}
        outT_ps = psum_w.tile([NAUG, BS], mybir.dt.float32, tag="outT")
        g_sgn = pe_gate(sgn_sb[:, 0, 0:1])

        def mm_group(part, m_sb, jt, ps, b0=0, b1=BS, war_sq=None):
            gates = [pe_gate(m_sb[:, jt, 0, 0:1]), g_xt[0]]
            if war_sq is None and jt >= 2:
                # psum slot last read by the square 2 rounds ago: observing
                # that square's output imports the needed ACT tick
                war_sq = sq_tiles[(part, jt - 2)]
            if war_sq is not None:
                gates.append(pe_gate(war_sq[:, 0:1]))
            for kt in range(KT):
                mm = nc.tensor.matmul(
                    ps,
                    lhsT=m_sb[:, jt, kt, :].bitcast(MM_DTYPE),
                    rhs=xt_sb[:, kt, b0:b1].bitcast(MM_DTYPE),
                    start=(kt == 0),
                    stop=(kt == KT - 1),
                )
                if kt == 0:
                    after(mm, gates)
                elif kt % XC == 0:
                    after(mm, [g_xt[kt // XC]])

        def postprocess(jt, ps_re_ap, ps_im_ap, b0, b1, is_first, is_last):
            # squares on ACT (sole PSUM reader), sum on DVE (sole probs writer)
            nb = b1 - b0
            sq_re = tmps.tile([128, nb], mybir.dt.float32, tag=f"sq_re{b0}")
            sq_im = tmps.tile([128, nb], mybir.dt.float32, tag=f"sq_im{b0}")
            nc.scalar.activation(
                out=sq_re, in_=ps_re_ap,
                func=mybir.ActivationFunctionType.Square,
            )
            nc.scalar.activation(
                out=sq_im, in_=ps_im_ap,
                func=mybir.ActivationFunctionType.Square,
            )
            sq_tiles[("re", jt)] = sq_re
            sq_tiles[("im", jt)] = sq_im
            nc.vector.tensor_add(probs_sb[:, jt, b0:b1], sq_re, sq_im)

            # fold this jt's probs into the signs contraction right away:
            # signs stationary (11-col weight load is ~free), probs moving.
            mo = nc.tensor.matmul(
                outT_ps[:, b0:b1],
                lhsT=sgn_sb[:, jt, :],
                rhs=probs_sb[:, jt, b0:b1],
                start=is_first,
                stop=is_last,
                skip_group_check=True,
            )
            if is_first:
                after(mo, [g_sgn])

        H = BS // 2
        for jt in range(JT):
            ps_re = psum.tile([128, BS], mybir.dt.float32, tag="ps_re")
            mm_group("re", mre_sb, jt, ps_re)
            if jt < JT - 1:
                ps_im = psum.tile([128, BS], mybir.dt.float32, tag="ps_im")
                mm_group("im", mim_sb, jt, ps_im)
                postprocess(jt, ps_re[:, :], ps_im[:, :], 0, BS, jt == 0, False)
            else:
                # split the last im round into separate half-batch PSUM
                # groups (separate banks) so the first half's squares/adds/
                # signs-fold pipeline under the second half's matmuls,
                # shortening the serial tail chain
                ps_im_a = psum.tile([128, H], mybir.dt.float32, tag="ps_im")
                mm_group("im", mim_sb, jt, ps_im_a, 0, H)
                ps_im_b = psum.tile([128, H], mybir.dt.float32, tag="ps_im")
                mm_group("im", mim_sb, jt, ps_im_b, H, BS,
                         war_sq=sq_tiles[("im", jt - 1)])
                postprocess(jt, ps_re[:, 0:H], ps_im_a[:, :], 0, H,
                            False, False)
                postprocess(jt, ps_re[:, H:BS], ps_im_b[:, :], H, BS,
                            False, True)

        # ---- transpose [11, BS] -> 4x [128, 11], then normalize ----
        outT_sb = singles.tile([NAUG, BS], mybir.dt.float32, tag="outT")
        nc.vector.tensor_copy(out=outT_sb, in_=outT_ps)
        g_outT = pe_gate(outT_sb[:, 0:1])
        g_ident = pe_gate(ident_sb[:, 0:1])
        for bc in range(BS // 128):
            gates_o = [g_outT, g_ident]
            if bc >= 2:
                # ps_o slot last read by the DVE mul two rounds ago
                gates_o.append(pe_gate(out_all[:, bc - 2, 0:1]))
            ps_o = psum_o.tile([128, NAUG], mybir.dt.float32, tag="ps_o")
            mm = nc.tensor.matmul(
                ps_o,
                lhsT=outT_sb[:, bc * 128:(bc + 1) * 128],
                rhs=ident_sb[:],
                start=True,
                stop=True,
            )
            after(mm, gates_o)
            recip = tmps.tile([128, 1], mybir.dt.float32, tag="recip")
            nc.vector.reciprocal(out=recip, in_=ps_o[:, N_QUBITS:N_QUBITS + 1])
            nc.vector.tensor_scalar_mul(out_all[:, bc, :], ps_o[:, 0:N_QUBITS],
                                        recip)

        # SWDGE stores (separate semaphore lanes from the HWDGE loads, so
        # the only wait is the DVE producer); two chunks so the first store
        # overlaps the last chunk's transpose/normalize work
        out_r = out_d.rearrange("(c p) q -> p c q", p=128)
        nc.gpsimd.dma_start(out=out_r[:, 0:2, :], in_=out_all[:, 0:2, :])
        nc.gpsimd.dma_start(out=out_r[:, 2:4, :], in_=out_all[:, 2:4, :])

    return nc


def _get_nc():
    global _CACHED_NC
    if _CACHED_NC is None:
        _CACHED_NC = _build_bass()
    return _CACHED_NC


# ----------------------------------------------------------------- entrypoint

def kernel(x, weights, weights_1, weights_2, _trace=False):
    x = np.asarray(x, dtype=np.float32)
    xf = x.reshape(B, DIM)

    M = _build_circuit_matrix(weights)
    mre_pack = _pack_m(M.real.astype(np.float32))
    mim_pack = _pack_m(M.imag.astype(np.float32))
    sgn_pack = _pack_k_major(_signs_aug())

    in_maps = []
    for c in range(N_CORES):
        shard = xf[c * BS:(c + 1) * BS]              # (BS, DIM)
        xt = np.ascontiguousarray(shard.T)           # (DIM, BS)
        xt_pack = _pack_k_major(xt)                  # (128, KT, BS)
        in_maps.append({
            "xt": xt_pack,
            "m_re": mre_pack,
            "m_im": mim_pack,
            "sgn": sgn_pack,
            "ident": np.eye(NAUG, dtype=np.float32),
        })

    nc = _get_nc()
    res = run_bass_kernel_spmd(nc, in_maps, core_ids=list(range(N_CORES)),
                               trace=_trace)
    out = np.concatenate([r["out"] for r in res.results], axis=0)
    if _trace:
        kernel.last_exec_time_ns = res.exec_time_ns
        kernel.last_results = res
    return out.astype(np.float32)
